# revision 16
# baseline (speedup 1.0000x reference)
"""Fused conv3x3 (pad bottom/right) -> +bias -> *scale -> maxpool2x2 -> clamp[0,1]
Trainium2 Bass kernel, data-parallel over batch: 32 images -> 4 per NeuronCore x 8 cores.

Per-core strategy (fp32 end to end):
  - Superblock = 16 output rows of one image. SBUF slab [108, 514]:
      partitions 0..53  = kw=0 plane: (ic, dr) ic*18+dr -> x[img, ic, h0+dr, :]
      partitions 54..107= kw=1 plane: same rows shifted left by 1 col (GpSimd copy)
    kw=2 handled as a col-shifted view of the kw0 plane (2nd accumulating matmul).
  - Two matmul streams: E (even output rows, M=(oc,j')=oc*8+j' -> row 2j') and
    O (odd rows). Each stream: matmul A (K=108, kw0+kw1) + matmul B (K=54, kw2),
    N=512, accumulated in PSUM. Stationaries precomputed on host (scale folded).
  - Groups of 2 superblocks share PSUM tiles [128, 1024] (2 banks each).
    Epilogue: ACT: o' = relu(O + bias); DVE: r = (E + bias) max o' (row pool),
    w = max(r even cols, r odd cols) (col pool), out = clamp(w, 0, 1).
  - Output [128=(oc,j'), 512] -> 2 DMAs to y[img, :, hp:hp+8, :].
"""
import sys

for _p in ("/opt/trn_rl_repo", "/root/.axon_site/_ro/trn_rl_repo"):
    if _p not in sys.path:
        sys.path.append(_p)

import numpy as np
from contextlib import ExitStack

import concourse.bass as bass
import concourse.tile as tile
from concourse import bacc, mybir
from concourse.bass_utils import run_bass_kernel_spmd

IC, OC = 3, 16
H = W = 512
SB = 16           # output rows per superblock
NDR = SB + 2      # 18 input rows per superblock
KA = 2 * IC * NDR  # 108
KB = IC * NDR      # 54
IMGS = 4           # images per core
N_CORES = 8
F32 = mybir.dt.float32


def build_stationaries(conv_weight, conv_bias, scale):
    """Host-side: fold scale into weights, build the 4 stationary matrices and
    the per-partition bias vector."""
    Wp = (conv_weight * scale.reshape(OC, 1, 1, 1)).astype(np.float32)
    bias_s = (conv_bias * scale.reshape(OC)).astype(np.float32)
    bias128 = np.repeat(bias_s, 8).astype(np.float32).reshape(128, 1)

    outs = {}
    for parity, nm in ((0, "e"), (1, "o")):
        wa = np.zeros((KA, 128), np.float32)
        wb = np.zeros((KB, 128), np.float32)
        # vectorized fill
        for j in range(8):
            r = 2 * j + parity
            for kh in range(3):
                dr = r + kh
                if dr >= NDR:
                    continue
                for ic in range(IC):
                    p = ic * NDR + dr
                    # m = oc*8 + j for all oc
                    m = np.arange(OC) * 8 + j
                    wa[p, m] = Wp[:, ic, kh, 0]
                    wa[KB + p, m] = Wp[:, ic, kh, 1]
                    wb[p, m] = Wp[:, ic, kh, 2]
        outs["wa_" + nm] = wa
        outs["wb_" + nm] = wb
    # single concatenated stationary tensor [KA, 512]:
    #   cols   0:128 = wa_e, 128:256 = wb_e (rows 0:54), 256:384 = wa_o,
    #   cols 384:512 = wb_o (rows 0:54)
    wcat = np.zeros((KA, 512), np.float32)
    wcat[:, 0:128] = outs["wa_e"]
    wcat[0:KB, 128:256] = outs["wb_e"]
    wcat[:, 256:384] = outs["wa_o"]
    wcat[0:KB, 384:512] = outs["wb_o"]
    return {"wcat": wcat, "bias128": bias128}


def build_nc():
    # Bacc (not raw Bass): its compile() pass splits multi-sem waits
    # (move_matmul_waits_to_ldweights + generate_event_semaphores) to satisfy
    # the 1-wait-per-instruction HW constraint.
    nc = bacc.Bacc(None, name="conv_pool_clamp")
    # x is zero-padded on the host by 2 rows/cols (bottom/right), so no
    # in-kernel memsets are ever needed.
    x4 = nc.dram_tensor("x4", [IMGS, IC, H + 2, W + 2], F32, kind="ExternalInput")
    wcat_d = nc.dram_tensor("wcat", [KA, 512], F32, kind="ExternalInput")
    bias_d = nc.dram_tensor("bias128", [128, 1], F32, kind="ExternalInput")
    y = nc.dram_tensor("y", [IMGS, OC, 256, 256], F32, kind="ExternalOutput")

    with tile.TileContext(nc) as tc, ExitStack() as ctx:
        consts = ctx.enter_context(tc.tile_pool(name="consts", bufs=1))
        slabs = ctx.enter_context(tc.tile_pool(name="slabs", bufs=4))
        psums = ctx.enter_context(tc.tile_pool(name="psums", bufs=2, space="PSUM"))
        work = ctx.enter_context(tc.tile_pool(name="work", bufs=2))

        wcat = consts.tile([KA, 512], F32)
        bias_t = consts.tile([128, 1], F32)
        nc.sync.dma_start(out=wcat, in_=wcat_d[:, :])
        nc.sync.dma_start(out=bias_t, in_=bias_d[:, :])
        wa_e = wcat[:, 0:128]
        wb_e = wcat[0:KB, 128:256]
        wa_o = wcat[:, 256:384]
        wb_o = wcat[0:KB, 384:512]

        for img in range(IMGS):
            for h0 in range(0, H, 2 * SB):  # group of 2 superblocks
                psum_e = psums.tile([128, 1024], F32, tag="pe")
                psum_o = psums.tile([128, 1024], F32, tag="po")
                for sbi in range(2):
                    h = h0 + SB * sbi
                    slab = slabs.tile([KA, 514], F32, tag="slab")
                    # single load: kw0 plane <- 18 padded rows x 3 channels.
                    # Shapes differ but DMA pairs elements by linearized order:
                    # dst (p=(ic,dr), c) matches src (ic, r, c).
                    nc.sync.dma_start(
                        out=slab[0:KB, 0:514], in_=x4[img, :, h: h + NDR, :]
                    )
                    # kw1 plane = kw0 plane shifted left by one column.
                    # Compute engines are partition-locked, so the cross-partition
                    # shift goes through an SBUF->SBUF DMA.
                    nc.gpsimd.dma_start(out=slab[KB:KA, 0:512], in_=slab[0:KB, 1:513])

                    pe = psum_e[:, 512 * sbi: 512 * (sbi + 1)]
                    po = psum_o[:, 512 * sbi: 512 * (sbi + 1)]
                    # B (kw2, reads only the kw0 plane -> depends only on the
                    # load DMA) opens each accumulation group; A (kw0+kw1)
                    # additionally depends on the shift DMA. This spreads the
                    # sync waits across instructions (HW wait-slot limit).
                    nc.tensor.matmul(pe, wb_e, slab[0:KB, 2:514], start=True, stop=False)
                    nc.tensor.matmul(pe, wa_e, slab[0:KA, 0:512], start=False, stop=True)
                    nc.tensor.matmul(po, wb_o, slab[0:KB, 2:514], start=True, stop=False)
                    nc.tensor.matmul(po, wa_o, slab[0:KA, 0:512], start=False, stop=True)

                # epilogue over the 2-superblock group
                o_sb = work.tile([128, 1024], F32, tag="osb")
                nc.scalar.activation(
                    out=o_sb, in_=psum_o,
                    func=mybir.ActivationFunctionType.Relu,
                    bias=bias_t, scale=1.0,
                )
                r_sb = work.tile([128, 1024], F32, tag="rsb")
                nc.vector.scalar_tensor_tensor(
                    out=r_sb, in0=psum_e, scalar=bias_t, in1=o_sb,
                    op0=mybir.AluOpType.add, op1=mybir.AluOpType.max,
                )
                r3 = r_sb.rearrange("p (a b) -> p a b", b=2)
                w_sb = work.tile([128, 512], F32, tag="wsb")
                nc.vector.tensor_max(w_sb, r3[:, :, 0], r3[:, :, 1])
                out_sb = work.tile([128, 512], F32, tag="outsb")
                nc.vector.tensor_scalar(
                    out_sb, w_sb, 0.0, 1.0,
                    mybir.AluOpType.max, mybir.AluOpType.min,
                )
                for half in range(2):
                    hp = h0 // 2 + 8 * half
                    # dst dims (oc, j, c) iterate in the same order as src (p=(oc,j), c)
                    nc.sync.dma_start(
                        out=y[img, :, hp: hp + 8, :],
                        in_=out_sb[:, 256 * half: 256 * (half + 1)],
                    )
    nc.finalize()  # Bacc: reg alloc + wait-splitting passes
    return nc


_NC_CACHE = None


def _get_nc():
    global _NC_CACHE
    if _NC_CACHE is None:
        _NC_CACHE = build_nc()
    return _NC_CACHE


def kernel(x, conv_weight, conv_bias, scale):
    x = np.ascontiguousarray(np.asarray(x, dtype=np.float32))
    conv_weight = np.asarray(conv_weight, dtype=np.float32)
    conv_bias = np.asarray(conv_bias, dtype=np.float32)
    scale = np.asarray(scale, dtype=np.float32)

    consts = build_stationaries(conv_weight, conv_bias, scale)
    xp = np.pad(x, ((0, 0), (0, 0), (0, 2), (0, 2)))  # zero-pad bottom/right
    in_maps = []
    for c in range(N_CORES):
        m = {"x4": np.ascontiguousarray(xp[IMGS * c: IMGS * (c + 1)])}
        m.update(consts)
        in_maps.append(m)

    res = run_bass_kernel_spmd(_get_nc(), in_maps, core_ids=list(range(N_CORES)))
    return np.concatenate([r["y"] for r in res.results], axis=0)


# revision 25
# speedup vs baseline: 1.0135x; 1.0135x over previous
"""Fused conv3x3 (pad bottom/right) -> +bias -> *scale -> maxpool2x2 -> clamp[0,1]
Trainium2 Bass kernel, data-parallel over batch: 32 images -> 4 per NeuronCore x 8 cores.

Per-core strategy (fp32 end to end):
  - Superblock = 16 output rows of one image. SBUF slab [108, 514]:
      partitions 0..53  = kw=0 plane: (ic, dr) ic*18+dr -> x[img, ic, h0+dr, :]
      partitions 54..107= kw=1 plane: same rows shifted left by 1 col (GpSimd copy)
    kw=2 handled as a col-shifted view of the kw0 plane (2nd accumulating matmul).
  - Two matmul streams: E (even output rows, M=(oc,j')=oc*8+j' -> row 2j') and
    O (odd rows). Each stream: matmul A (K=108, kw0+kw1) + matmul B (K=54, kw2),
    N=512, accumulated in PSUM. Stationaries precomputed on host (scale folded).
  - Groups of 2 superblocks share PSUM tiles [128, 1024] (2 banks each).
    Epilogue: ACT: o' = relu(O + bias); DVE: r = (E + bias) max o' (row pool),
    w = max(r even cols, r odd cols) (col pool), out = clamp(w, 0, 1).
  - Output [128=(oc,j'), 512] -> 2 DMAs to y[img, :, hp:hp+8, :].
"""
import sys

for _p in ("/opt/trn_rl_repo", "/root/.axon_site/_ro/trn_rl_repo"):
    if _p not in sys.path:
        sys.path.append(_p)

import numpy as np
from contextlib import ExitStack

import concourse.bass as bass
import concourse.tile as tile
from concourse import bacc, mybir
from concourse.bass_utils import run_bass_kernel_spmd

IC, OC = 3, 16
H = W = 512
SB = 16           # output rows per superblock
NDR = SB + 2      # 18 input rows per superblock
KA = 2 * IC * NDR  # 108
KB = IC * NDR      # 54
IMGS = 4           # images per core
N_CORES = 8
F32 = mybir.dt.float32
import os
USE_SHIFT_DMA = os.environ.get("USE_SHIFT_DMA", "0") == "1"
PAD_COLS = int(os.environ.get("PAD_COLS", "4"))


def build_stationaries(conv_weight, conv_bias, scale):
    """Host-side: fold scale into weights, build the 4 stationary matrices and
    the per-partition bias vector."""
    Wp = (conv_weight * scale.reshape(OC, 1, 1, 1)).astype(np.float32)
    bias_s = (conv_bias * scale.reshape(OC)).astype(np.float32)
    bias128 = np.repeat(bias_s, 8).astype(np.float32).reshape(128, 1)

    outs = {}
    for parity, nm in ((0, "e"), (1, "o")):
        wa = np.zeros((KA, 128), np.float32)
        wb = np.zeros((KB, 128), np.float32)
        # vectorized fill
        for j in range(8):
            r = 2 * j + parity
            for kh in range(3):
                dr = r + kh
                if dr >= NDR:
                    continue
                for ic in range(IC):
                    p = ic * NDR + dr
                    # m = oc*8 + j for all oc
                    m = np.arange(OC) * 8 + j
                    wa[p, m] = Wp[:, ic, kh, 0]
                    wa[KB + p, m] = Wp[:, ic, kh, 1]
                    wb[p, m] = Wp[:, ic, kh, 2]
        outs["wa_" + nm] = wa
        outs["wb_" + nm] = wb
    # single concatenated stationary tensor [KA, 512]:
    #   cols   0:128 = wa_e, 128:256 = wb_e (rows 0:54), 256:384 = wa_o,
    #   cols 384:512 = wb_o (rows 0:54)
    wcat = np.zeros((KA, 512), np.float32)
    wcat[:, 0:128] = outs["wa_e"]
    wcat[0:KB, 128:256] = outs["wb_e"]
    wcat[:, 256:384] = outs["wa_o"]
    wcat[0:KB, 384:512] = outs["wb_o"]
    return {"wcat": wcat, "bias128": bias128}


def build_nc():
    # Bacc (not raw Bass): its compile() pass splits multi-sem waits
    # (move_matmul_waits_to_ldweights + generate_event_semaphores) to satisfy
    # the 1-wait-per-instruction HW constraint.
    nc = bacc.Bacc(None, name="conv_pool_clamp")
    # x is zero-padded on the host (2 rows bottom, 4 cols right), so no
    # in-kernel memsets are ever needed. The odd row pitch (516 != 514 read
    # width) keeps each (kw, ic, row) chunk a separate DMA descriptor, which
    # spreads the load across all 16 SDMA engines.
    x4 = nc.dram_tensor("x4", [IMGS, IC, H + 2, W + PAD_COLS], F32, kind="ExternalInput")
    wcat_d = nc.dram_tensor("wcat", [KA, 512], F32, kind="ExternalInput")
    bias_d = nc.dram_tensor("bias128", [128, 1], F32, kind="ExternalInput")
    y = nc.dram_tensor("y", [IMGS, OC, 256, 256], F32, kind="ExternalOutput")

    with tile.TileContext(nc) as tc, ExitStack() as ctx:
        consts = ctx.enter_context(tc.tile_pool(name="consts", bufs=1))
        slabs = ctx.enter_context(tc.tile_pool(name="slabs", bufs=4))
        psums = ctx.enter_context(tc.tile_pool(name="psums", bufs=2, space="PSUM"))
        work = ctx.enter_context(tc.tile_pool(name="work", bufs=2))

        wcat = consts.tile([KA, 512], F32)
        bias_t = consts.tile([128, 1], F32)
        nc.sync.dma_start(out=wcat, in_=wcat_d[:, :])
        nc.sync.dma_start(out=bias_t, in_=bias_d[:, :])
        wa_e = wcat[:, 0:128]
        wb_e = wcat[0:KB, 128:256]
        wa_o = wcat[:, 256:384]
        wb_o = wcat[0:KB, 384:512]

        for img in range(IMGS):
            for h0 in range(0, H, 2 * SB):  # group of 2 superblocks
                psum_e = psums.tile([128, 1024], F32, tag="pe")
                psum_o = psums.tile([128, 1024], F32, tag="po")
                for sbi in range(2):
                    h = h0 + SB * sbi
                    slab = slabs.tile([KA, 514], F32, tag="slab")
                    # Load BOTH kw planes straight from (padded) HBM (DMA APs
                    # are limited to 3 dims, so one DMA per kw plane):
                    # partition p = kw*54 + ic*18 + dr <- x4[img, ic, h+dr, kw:kw+514]
                    nc.sync.dma_start(
                        out=slab[0:KB, 0:514], in_=x4[img, :, h: h + NDR, 0:514]
                    )
                    if USE_SHIFT_DMA:
                        nc.gpsimd.dma_start(
                            out=slab[KB:KA, 0:512], in_=slab[0:KB, 1:513]
                        )
                    else:
                        nc.sync.dma_start(
                            out=slab[KB:KA, 0:514], in_=x4[img, :, h: h + NDR, 1:515]
                        )

                    pe = psum_e[:, 512 * sbi: 512 * (sbi + 1)]
                    po = psum_o[:, 512 * sbi: 512 * (sbi + 1)]
                    # B (kw2, reads only the kw0 plane -> depends only on the
                    # load DMA) opens each accumulation group; A (kw0+kw1)
                    # additionally depends on the shift DMA. This spreads the
                    # sync waits across instructions (HW wait-slot limit).
                    nc.tensor.matmul(pe, wb_e, slab[0:KB, 2:514], start=True, stop=False)
                    nc.tensor.matmul(pe, wa_e, slab[0:KA, 0:512], start=False, stop=True)
                    nc.tensor.matmul(po, wb_o, slab[0:KB, 2:514], start=True, stop=False)
                    nc.tensor.matmul(po, wa_o, slab[0:KA, 0:512], start=False, stop=True)

                # epilogue over the 2-superblock group
                o_sb = work.tile([128, 1024], F32, tag="osb")
                nc.scalar.activation(
                    out=o_sb, in_=psum_o,
                    func=mybir.ActivationFunctionType.Relu,
                    bias=bias_t, scale=1.0,
                )
                r_sb = work.tile([128, 1024], F32, tag="rsb")
                nc.vector.scalar_tensor_tensor(
                    out=r_sb, in0=psum_e, scalar=bias_t, in1=o_sb,
                    op0=mybir.AluOpType.add, op1=mybir.AluOpType.max,
                )
                r3 = r_sb.rearrange("p (a b) -> p a b", b=2)
                w_sb = work.tile([128, 512], F32, tag="wsb")
                nc.vector.tensor_max(w_sb, r3[:, :, 0], r3[:, :, 1])
                out_sb = work.tile([128, 512], F32, tag="outsb")
                nc.vector.tensor_scalar(
                    out_sb, w_sb, 0.0, 1.0,
                    mybir.AluOpType.max, mybir.AluOpType.min,
                )
                for half in range(2):
                    hp = h0 // 2 + 8 * half
                    # dst dims (oc, j, c) iterate in the same order as src (p=(oc,j), c)
                    nc.sync.dma_start(
                        out=y[img, :, hp: hp + 8, :],
                        in_=out_sb[:, 256 * half: 256 * (half + 1)],
                    )
    nc.finalize()  # Bacc: reg alloc + wait-splitting passes
    return nc


_NC_CACHE = None


def _get_nc():
    global _NC_CACHE
    if _NC_CACHE is None:
        _NC_CACHE = build_nc()
    return _NC_CACHE


def kernel(x, conv_weight, conv_bias, scale):
    x = np.ascontiguousarray(np.asarray(x, dtype=np.float32))
    conv_weight = np.asarray(conv_weight, dtype=np.float32)
    conv_bias = np.asarray(conv_bias, dtype=np.float32)
    scale = np.asarray(scale, dtype=np.float32)

    consts = build_stationaries(conv_weight, conv_bias, scale)
    xp = np.pad(x, ((0, 0), (0, 0), (0, 2), (0, PAD_COLS)))  # zero-pad bottom/right
    in_maps = []
    for c in range(N_CORES):
        m = {"x4": np.ascontiguousarray(xp[IMGS * c: IMGS * (c + 1)])}
        m.update(consts)
        in_maps.append(m)

    res = run_bass_kernel_spmd(_get_nc(), in_maps, core_ids=list(range(N_CORES)))
    return np.concatenate([r["y"] for r in res.results], axis=0)
